# revision 4
# baseline (speedup 1.0000x reference)
"""Self-contained Trainium2 Bass kernel for a single attention head.

Problem: B=8, S=2048, E=1024, D=64 (fp32 in/out).
  q = query @ Wq.T + bq ; k, v likewise
  out = softmax(mask(q @ k.T / sqrt(D))) @ v
  mask = query_mask[:, :, None] * key_mask[:, None, :] (query_mask is all-ones
  per the problem spec: fill="ones").

Sharding: pure data-parallel, one batch element per NeuronCore (8 cores).

Per-core algorithm (all layouts transposed so the contraction dim lands on
SBUF partitions and the key-position dim lands on partitions for the softmax):
  - host supplies qT/kT/vT = x[b].T as [E, S] fp16
  - qT_sb[d, i] = Wq @ queryT (+bq)   [64, 2048] fp16   (weights stationary)
  - kT_sb[d, j], vT_sb[d, j] likewise
  - X[j] = [ v[j*128:(j+1)*128, :] | ones ]  [128, 65] fp16 via PE transpose
  - per (i-half h, key tile j):
      sT = kT_sb[:, j-tile].T @ qT_sb[:, h-half]      [128, 1024] f32 PSUM
      p  = exp(sT * 0.125 + mask_bias[j])             [128, 1024] fp16
          (mask_bias = 0 for live keys, -30000 for masked -> exp == 0,
           which applies the key mask exactly; no row-max subtraction is
           needed: |s|/8 <= ~6 so exp <= ~300, far from overflow)
      num[d, i] (+= over j) = X[j].T @ p              [65, 1024] f32 PSUM
          (row 64 accumulates the softmax denominator)
  - transpose num back in 128-col chunks, divide by the denominator column,
    DMA out f32 [2048, 64].
"""

import os

import numpy as np

import concourse.bass as bass
import concourse.mybir as mybir
import concourse.tile as tile
from concourse import bacc
from concourse.bass_utils import run_bass_kernel_spmd
from concourse.masks import make_identity

FP16 = mybir.dt.float16
F32 = mybir.dt.float32

N_CORES = 8
B, S, E, D = 8, 2048, 1024, 64
P = 128
NE = E // P            # 8 contraction tiles
NJ = S // P            # 16 key tiles
NH = 2                 # i halves (PSUM capacity)
HI = S // NH           # 1024 query positions per half
NC = 512               # matmul free-dim chunk (one PSUM bank of f32)
SCALE = 1.0 / np.sqrt(np.float32(D))
MASK_NEG = -30000.0


def _build(tc: tile.TileContext, ins: dict, out_d: bass.AP, ctx):
    nc = tc.nc
    qT_d, kT_d, vT_d = ins["qT"], ins["kT"], ins["vT"]
    wq_d, wk_d, wv_d = ins["wq"], ins["wk"], ins["wv"]
    bq_d, bk_d, bv_d = ins["bq"], ins["bk"], ins["bv"]
    mb_d = ins["mask_bias"]

    consts = ctx.enter_context(tc.tile_pool(name="consts", bufs=1))
    stage = ctx.enter_context(tc.tile_pool(name="stage", bufs=16))
    proj = ctx.enter_context(tc.tile_pool(name="proj", bufs=1))
    xpool = ctx.enter_context(tc.tile_pool(name="xpool", bufs=16))
    ppool = ctx.enter_context(tc.tile_pool(name="ppool", bufs=16))
    fin = ctx.enter_context(tc.tile_pool(name="fin", bufs=4))
    ps_mm = ctx.enter_context(tc.tile_pool(name="ps_mm", bufs=2, space="PSUM"))
    ps_sm = ctx.enter_context(tc.tile_pool(name="ps_sm", bufs=2, space="PSUM"))
    ps_acc = ctx.enter_context(tc.tile_pool(name="ps_acc", bufs=1, space="PSUM"))

    # --- constants -------------------------------------------------------
    wq = consts.tile([P, NE * D], FP16, tag="wq")
    wk = consts.tile([P, NE * D], FP16, tag="wk")
    wv = consts.tile([P, NE * D], FP16, tag="wv")
    bq = consts.tile([D, 1], F32, tag="bq")
    bk = consts.tile([D, 1], F32, tag="bk")
    bv = consts.tile([D, 1], F32, tag="bv")
    mb = consts.tile([P, NJ], F32, tag="mb")
    ident16 = consts.tile([P, P], FP16, tag="ident16")
    ident32 = consts.tile([P, P], F32, tag="ident32")
    warm = consts.tile([P, 16], F32, tag="warm")
    for t, d_ in ((wq, wq_d), (wk, wk_d), (wv, wv_d), (bq, bq_d), (bk, bk_d),
                  (bv, bv_d), (mb, mb_d)):
        nc.sync.dma_start(out=t[:], in_=d_[:])
    make_identity(nc, ident16[:])
    make_identity(nc, ident32[:])
    # Load the ACT exp table at t~0 instead of before the first real exp.
    nc.vector.memset(warm[:], 0.0)
    nc.scalar.activation(warm[:], warm[:], mybir.ActivationFunctionType.Exp)

    # persistent projected tensors
    qT_sb = proj.tile([D, S], FP16, tag="qT_sb")
    kT_sb = proj.tile([D, S], FP16, tag="kT_sb")
    vT_sb = proj.tile([D, S], FP16, tag="vT_sb")

    def project(dst, w, bias_ap, src_d, half):
        """dst[:, half*HI:+HI] (fp16) = (W @ srcT)[:, half] + bias."""
        ps = ps_mm.tile([P, HI], F32, tag="ps_mm")
        st = []
        for e in range(NE):
            t = stage.tile([P, HI], FP16, tag=f"stage_{src_d.tensor.name}")
            nc.sync.dma_start(out=t[:], in_=src_d[e * P:(e + 1) * P,
                                                  half * HI:(half + 1) * HI])
            st.append(t)
        for c in range(HI // NC):
            for e in range(NE):
                nc.tensor.matmul(
                    ps[:D, c * NC:(c + 1) * NC],
                    wq_lhsT(w, e),
                    st[e][:, c * NC:(c + 1) * NC],
                    start=(e == 0), stop=(e == NE - 1),
                )
        nc.vector.tensor_scalar_add(
            dst[:, half * HI:(half + 1) * HI], ps[:D, :], bias_ap)

    def wq_lhsT(w, e):
        return w[:, e * D:(e + 1) * D]

    # q half 0 first (unblocks the h=0 score loop), then k fully, then v,
    # then q half 1.  DMA queue order follows emission order.
    project(qT_sb, wq, bq[:], qT_d, 0)
    project(kT_sb, wk, bk[:], kT_d, 0)
    project(kT_sb, wk, bk[:], kT_d, 1)
    project(vT_sb, wv, bv[:], vT_d, 0)
    project(vT_sb, wv, bv[:], vT_d, 1)

    # X[j] = [v rows | ones column]  [128, 65] fp16
    xt = []
    for j in range(NJ):
        ps = ps_sm.tile([P, D], FP16, tag="ps_sm")
        nc.tensor.transpose(ps[:], vT_sb[:, j * P:(j + 1) * P],
                            ident16[:D, :D])
        x = xpool.tile([P, D + 1], FP16, tag="x")
        nc.vector.tensor_copy(x[:, :D], ps[:])
        nc.vector.memset(x[:, D:D + 1], 1.0)
        xt.append(x)

    project(qT_sb, wq, bq[:], qT_d, 1)

    # --- attention -------------------------------------------------------
    for h in range(NH):
        num = ps_acc.tile([D + 1, HI], F32, tag="num")
        pm = []
        for j in range(NJ):
            ssT = ps_mm.tile([P, HI], F32, tag="ps_mm")
            for c in range(HI // NC):
                nc.tensor.matmul(
                    ssT[:, c * NC:(c + 1) * NC],
                    kT_sb[:, j * P:(j + 1) * P],
                    qT_sb[:, h * HI + c * NC:h * HI + (c + 1) * NC],
                    start=True, stop=True,
                )
            p = ppool.tile([P, HI], FP16, tag="pm")
            nc.scalar.activation(p[:], ssT[:], mybir.ActivationFunctionType.Exp,
                                 bias=mb[:, j:j + 1], scale=float(SCALE))
            pm.append(p)
        for j in range(NJ):
            for c in range(HI // NC):
                nc.tensor.matmul(
                    num[:, c * NC:(c + 1) * NC],
                    xt[j][:],
                    pm[j][:, c * NC:(c + 1) * NC],
                    start=(j == 0), stop=(j == NJ - 1),
                )

        # finalize: transpose 128-col chunks back, normalize, store
        nsb = fin.tile([D + 1, HI], F32, tag="nsb")
        nc.vector.tensor_copy(nsb[:], num[:])
        for it in range(HI // P):
            pst = ps_sm.tile([P, D + 1], F32, tag="ps_sm")
            nc.tensor.transpose(pst[:], nsb[:, it * P:(it + 1) * P],
                                ident32[:D + 1, :D + 1])
            rec = fin.tile([P, 1], F32, tag="rec")
            nc.vector.reciprocal(rec[:], pst[:, D:D + 1])
            ot = fin.tile([P, D], F32, tag="ot")
            nc.vector.tensor_scalar_mul(ot[:], pst[:, :D], rec[:])
            r0 = h * HI + it * P
            nc.sync.dma_start(out=out_d[r0:r0 + P, :], in_=ot[:])


_COMPILED = None


def _get_compiled():
    global _COMPILED
    if _COMPILED is None:
        nc = bacc.Bacc("TRN2", target_bir_lowering=False, debug=False,
                       num_devices=N_CORES)
        ins = {
            "qT": nc.dram_tensor("qT", [E, S], FP16, kind="ExternalInput").ap(),
            "kT": nc.dram_tensor("kT", [E, S], FP16, kind="ExternalInput").ap(),
            "vT": nc.dram_tensor("vT", [E, S], FP16, kind="ExternalInput").ap(),
            "wq": nc.dram_tensor("wq", [P, NE * D], FP16, kind="ExternalInput").ap(),
            "wk": nc.dram_tensor("wk", [P, NE * D], FP16, kind="ExternalInput").ap(),
            "wv": nc.dram_tensor("wv", [P, NE * D], FP16, kind="ExternalInput").ap(),
            "bq": nc.dram_tensor("bq", [D, 1], F32, kind="ExternalInput").ap(),
            "bk": nc.dram_tensor("bk", [D, 1], F32, kind="ExternalInput").ap(),
            "bv": nc.dram_tensor("bv", [D, 1], F32, kind="ExternalInput").ap(),
            "mask_bias": nc.dram_tensor("mask_bias", [P, NJ], F32,
                                        kind="ExternalInput").ap(),
        }
        out_d = nc.dram_tensor("out", [S, D], F32, kind="ExternalOutput").ap()
        from contextlib import ExitStack
        with tile.TileContext(nc) as tc:
            with ExitStack() as ctx:
                _build(tc, ins, out_d, ctx)
        nc.compile()
        _COMPILED = nc
    return _COMPILED


def _reshape_w(w):
    # [D, E] -> [128, NE*D] with w_r[p, e*D + d] = w[d, e*128 + p]
    return np.ascontiguousarray(
        w.astype(np.float16).reshape(D, NE, P).transpose(2, 1, 0).reshape(P, NE * D))


LAST_RESULTS = None


def kernel(query, key, value, query_mask, key_mask, Wq, bq, Wk, bk, Wv, bv):
    global LAST_RESULTS
    query = np.asarray(query, dtype=np.float32)
    key = np.asarray(key, dtype=np.float32)
    value = np.asarray(value, dtype=np.float32)
    key_mask = np.asarray(key_mask)
    wq_r = _reshape_w(np.asarray(Wq, dtype=np.float32))
    wk_r = _reshape_w(np.asarray(Wk, dtype=np.float32))
    wv_r = _reshape_w(np.asarray(Wv, dtype=np.float32))
    bq_r = np.ascontiguousarray(np.asarray(bq, np.float32).reshape(D, 1))
    bk_r = np.ascontiguousarray(np.asarray(bk, np.float32).reshape(D, 1))
    bv_r = np.ascontiguousarray(np.asarray(bv, np.float32).reshape(D, 1))

    in_maps = []
    for c in range(N_CORES):
        mbias = np.where(np.asarray(key_mask[c]) == 0, np.float32(MASK_NEG),
                         np.float32(0.0)).reshape(NJ, P).T
        in_maps.append({
            "qT": np.ascontiguousarray(query[c].T).astype(np.float16),
            "kT": np.ascontiguousarray(key[c].T).astype(np.float16),
            "vT": np.ascontiguousarray(value[c].T).astype(np.float16),
            "wq": wq_r, "wk": wk_r, "wv": wv_r,
            "bq": bq_r, "bk": bk_r, "bv": bv_r,
            "mask_bias": np.ascontiguousarray(mbias),
        })

    nc = _get_compiled()
    res = run_bass_kernel_spmd(nc, in_maps, core_ids=list(range(N_CORES)))
    LAST_RESULTS = res
    return np.stack([res.results[c]["out"] for c in range(N_CORES)], axis=0)


# revision 8
# speedup vs baseline: 1.1857x; 1.1857x over previous
"""Self-contained Trainium2 Bass kernel for a single attention head.

Problem: B=8, S=2048, E=1024, D=64 (fp32 in/out).
  q = query @ Wq.T + bq ; k, v likewise
  out = softmax(mask(q @ k.T / sqrt(D))) @ v
  mask = query_mask[:, :, None] * key_mask[:, None, :]; query_mask is all-ones
  per the problem spec (fill="ones").

Sharding: pure data-parallel, one batch element per NeuronCore (8 cores).

Key ideas:
  - fp16 compute with fp32 PSUM accumulation (rel err ~7e-4 vs f32 ref;
    fp16 matmul streams 1 col/cycle vs 4 for fp32).
  - Host compacts away masked key columns (they contribute exactly 0 through
    exp(-inf)); S_k shrinks from 2048 to ~1100, padded to a multiple of 128.
    Pad columns get mask bias -30000 -> exp underflows to exactly 0.
  - Everything transposed so contractions sit on SBUF partitions and softmax's
    key dim sits on partitions: the key mask becomes a per-partition bias on
    the ACT exp (func(scale*x + bias)), and the softmax denominator falls out
    of the AV matmul as a 65th output row (X = [v | ones]).
  - No row-max subtraction: scores/sqrt(D) stay within +-~6, exp <= ~300.
  - Scores matmuls are K=64: row-packed two key-tiles per pass via
    tile_position (0,0)/(64,0) with qT/kT duplicated into partitions 64-127.
  - 4 large staged input DMAs issued on the (otherwise idle) GpSimd SWDGE
    path; tiny const + output DMAs on the Sync HWDGE ring.
"""

from contextlib import ExitStack

import numpy as np

import concourse.bass as bass
import concourse.mybir as mybir
import concourse.tile as tile
from concourse import bacc
from concourse.bass_utils import run_bass_kernel_spmd
from concourse.masks import make_identity

FP16 = mybir.dt.float16
F32 = mybir.dt.float32

N_CORES = 8
B, S, E, D = 8, 2048, 1024, 64
P = 128
NE = E // P            # 8 contraction tiles
NH = 2                 # i halves (PSUM capacity)
HI = S // NH           # 1024 query positions per half
NC = 512               # matmul free-dim chunk (one PSUM bank of f32)
SCALE = 1.0 / np.sqrt(np.float32(D))
MASK_NEG = -30000.0


def _chunks(total, step):
    out = []
    o = 0
    while o < total:
        out.append((o, min(step, total - o)))
        o += step
    return out


def _build(tc: tile.TileContext, ins: dict, out_d: bass.AP, ctx, sk2: int):
    nc = tc.nc
    nj = sk2 // P
    qst_d, kst_d, vst_d = ins["qst"], ins["kst"], ins["vst"]
    c16_d, c32_d = ins["c16"], ins["c32"]

    consts = ctx.enter_context(tc.tile_pool(name="consts", bufs=1))
    stage = ctx.enter_context(tc.tile_pool(name="stage", bufs=2))
    proj = ctx.enter_context(tc.tile_pool(name="proj", bufs=1))
    xpool = ctx.enter_context(tc.tile_pool(name="xpool", bufs=16))
    ppool = ctx.enter_context(tc.tile_pool(name="ppool", bufs=16))
    fin = ctx.enter_context(tc.tile_pool(name="fin", bufs=2))
    ps_mm = ctx.enter_context(tc.tile_pool(name="ps_mm", bufs=2, space="PSUM"))
    ps_sm = ctx.enter_context(tc.tile_pool(name="ps_sm", bufs=2, space="PSUM"))
    ps_acc = ctx.enter_context(tc.tile_pool(name="ps_acc", bufs=1, space="PSUM"))

    # --- staged inputs (4 big SWDGE DMAs; e-blocks side by side) --------
    qst = [stage.tile([P, NE * HI], FP16, tag="qst", bufs=2, name=f"qst{i}")
           for i in range(NH)]
    nc.gpsimd.dma_start(
        out=qst[0][:].rearrange("p (e s) -> p e s", e=NE),
        in_=qst_d.rearrange("(e p) s -> p e s", p=P)[:, :, 0:HI])
    kst = stage.tile([P, NE * sk2], FP16, tag="kst", bufs=1)
    nc.gpsimd.dma_start(
        out=kst[:].rearrange("p (e s) -> p e s", e=NE),
        in_=kst_d.rearrange("(e p) s -> p e s", p=P))
    vst = stage.tile([P, NE * sk2], FP16, tag="vst", bufs=1)
    nc.gpsimd.dma_start(
        out=vst[:].rearrange("p (e s) -> p e s", e=NE),
        in_=vst_d.rearrange("(e p) s -> p e s", p=P))
    nc.gpsimd.dma_start(
        out=qst[1][:].rearrange("p (e s) -> p e s", e=NE),
        in_=qst_d.rearrange("(e p) s -> p e s", p=P)[:, :, HI:S])

    # --- constants -------------------------------------------------------
    c16 = consts.tile([P, 3 * NE * D], FP16, tag="c16")
    c32 = consts.tile([P, nj + 3], F32, tag="c32")
    nc.sync.dma_start(out=c16[:], in_=c16_d[:])
    nc.sync.dma_start(out=c32[:], in_=c32_d[:])
    wq = c16[:, 0:NE * D]
    wk = c16[:, NE * D:2 * NE * D]
    wv = c16[:, 2 * NE * D:3 * NE * D]
    mb = c32[:, 0:nj]
    bq = c32[0:D, nj:nj + 1]
    bk = c32[0:D, nj + 1:nj + 2]
    bv = c32[0:D, nj + 2:nj + 3]

    ident16 = consts.tile([P, P], FP16, tag="ident16")
    ident32 = consts.tile([P, P], F32, tag="ident32")
    warm = consts.tile([P, 16], F32, tag="warm")
    make_identity(nc, ident16[:])
    make_identity(nc, ident32[:])
    nc.vector.memset(warm[:], 0.0)
    nc.scalar.activation(warm[:], warm[:], mybir.ActivationFunctionType.Exp)

    # persistent projected tensors (rows 64:128 duplicate rows 0:64 for the
    # row-packed score matmuls)
    qT_sb = proj.tile([P, S], FP16, tag="qT_sb")
    kT_sb = proj.tile([P, sk2], FP16, tag="kT_sb")
    vT_sb = proj.tile([D, sk2], FP16, tag="vT_sb")

    def project(dst, w, bias_ap, src, ncols, c0):
        """dst[0:64, c0:c0+ncols] = (W @ xT)[:, c0:c0+ncols] + bias."""
        for (po, pn) in _chunks(ncols, HI):
            ps = ps_mm.tile([P, HI], F32, tag="ps_mm")
            for (o, n) in _chunks(pn, NC):
                for e in range(NE):
                    nc.tensor.matmul(
                        ps[0:D, o:o + n],
                        w[:, e * D:(e + 1) * D],
                        src[:, e * ncols + po + o:e * ncols + po + o + n],
                        start=(e == 0), stop=(e == NE - 1),
                    )
            nc.vector.tensor_scalar_add(dst[0:D, c0 + po:c0 + po + pn],
                                        ps[0:D, 0:pn], bias_ap)

    # q half 0, k, v, q half 1 (matches DMA arrival order)
    project(qT_sb, wq, bq, qst[0][:], HI, 0)
    project(kT_sb, wk, bk, kst[:], sk2, 0)
    # duplicate into partitions 64:128 for row-packed scores
    nc.sync.dma_start(out=kT_sb[D:P, :], in_=kT_sb[0:D, :])
    project(vT_sb, wv, bv, vst[:], sk2, 0)

    # X[j] = [v rows | ones column]  [128, 65] fp16
    xt = []
    for j in range(nj):
        ps = ps_sm.tile([P, D], FP16, tag="ps_sm")
        nc.tensor.transpose(ps[:], vT_sb[:, j * P:(j + 1) * P],
                            ident16[0:D, 0:D])
        x = xpool.tile([P, D + 1], FP16, tag="x")
        nc.vector.tensor_copy(x[:, 0:D], ps[:])
        nc.vector.memset(x[:, D:D + 1], 1.0)
        xt.append(x)

    project(qT_sb, wq, bq, qst[1][:], HI, HI)
    nc.sync.dma_start(out=qT_sb[D:P, :], in_=qT_sb[0:D, :])

    # --- attention -------------------------------------------------------
    for h in range(NH):
        num = ps_acc.tile([D + 1, HI], F32, tag="num")
        pm = []
        for j0 in range(0, nj, 2):
            pair = [j0] + ([j0 + 1] if j0 + 1 < nj else [])
            pss = []
            for i, j in enumerate(pair):
                ssT = ps_mm.tile([P, HI], F32, tag="ps_mm")
                lo = i * D
                for c in range(HI // NC):
                    nc.tensor.matmul(
                        ssT[:, c * NC:(c + 1) * NC],
                        kT_sb[lo:lo + D, j * P:(j + 1) * P],
                        qT_sb[lo:lo + D, h * HI + c * NC:h * HI + (c + 1) * NC],
                        start=True, stop=True,
                        tile_position=(lo, 0),
                    )
                pss.append(ssT)
            for i, j in enumerate(pair):
                p = ppool.tile([P, HI], FP16, tag="pm")
                nc.scalar.activation(p[:], pss[i][:],
                                     mybir.ActivationFunctionType.Exp,
                                     bias=mb[:, j:j + 1], scale=float(SCALE))
                pm.append(p)
        for j in range(nj):
            for c in range(HI // NC):
                nc.tensor.matmul(
                    num[:, c * NC:(c + 1) * NC],
                    xt[j][:],
                    pm[j][:, c * NC:(c + 1) * NC],
                    start=(j == 0), stop=(j == nj - 1),
                )

        # finalize: transpose 128-col chunks back, normalize, store
        nsb = fin.tile([D + 1, HI], F32, tag="nsb")
        nc.vector.tensor_copy(nsb[:], num[:])
        osb = fin.tile([P, (HI // P) * D], F32, tag="osb")
        for it in range(HI // P):
            pst = ps_sm.tile([P, D + 1], F32, tag="ps_sm")
            nc.tensor.transpose(pst[:], nsb[:, it * P:(it + 1) * P],
                                ident32[0:D + 1, 0:D + 1])
            rec = fin.tile([P, 1], F32, tag="rec")
            nc.vector.reciprocal(rec[:], pst[:, D:D + 1])
            nc.vector.tensor_scalar_mul(osb[:, it * D:(it + 1) * D],
                                        pst[:, 0:D], rec[:])
        nc.sync.dma_start(
            out=out_d[h * HI:(h + 1) * HI, :].rearrange("(t p) d -> p t d", p=P),
            in_=osb[:].rearrange("p (t d) -> p t d", d=D))


_COMPILED = {}


def _get_compiled(sk2: int):
    if sk2 not in _COMPILED:
        nj = sk2 // P
        nc = bacc.Bacc("TRN2", target_bir_lowering=False, debug=False,
                       num_devices=N_CORES)
        ins = {
            "qst": nc.dram_tensor("qst", [E, S], FP16, kind="ExternalInput").ap(),
            "kst": nc.dram_tensor("kst", [E, sk2], FP16, kind="ExternalInput").ap(),
            "vst": nc.dram_tensor("vst", [E, sk2], FP16, kind="ExternalInput").ap(),
            "c16": nc.dram_tensor("c16", [P, 3 * NE * D], FP16,
                                  kind="ExternalInput").ap(),
            "c32": nc.dram_tensor("c32", [P, nj + 3], F32,
                                  kind="ExternalInput").ap(),
        }
        out_d = nc.dram_tensor("out", [S, D], F32, kind="ExternalOutput").ap()
        with tile.TileContext(nc) as tc:
            with ExitStack() as ctx:
                _build(tc, ins, out_d, ctx, sk2)
        nc.compile()
        _COMPILED[sk2] = nc
    return _COMPILED[sk2]


LAST_RESULTS = None


def kernel(query, key, value, query_mask, key_mask, Wq, bq, Wk, bk, Wv, bv):
    global LAST_RESULTS
    query = np.asarray(query, dtype=np.float32)
    key = np.asarray(key, dtype=np.float32)
    value = np.asarray(value, dtype=np.float32)
    key_mask = np.asarray(key_mask)

    # compact masked keys away (they contribute exactly zero)
    keeps = [np.nonzero(key_mask[c] != 0)[0] for c in range(N_CORES)]
    nk_max = max(len(kp) for kp in keeps)
    sk2 = max(P, int(np.ceil(nk_max / P)) * P)
    sk2 = min(sk2, S)
    nj = sk2 // P

    w16 = np.concatenate(
        [np.asarray(w, np.float32).astype(np.float16)
         .reshape(D, NE, P).transpose(2, 1, 0).reshape(P, NE * D)
         for w in (Wq, Wk, Wv)], axis=1)
    c32 = np.zeros((P, nj + 3), np.float32)
    c32[0:D, nj] = np.asarray(bq, np.float32).reshape(D)
    c32[0:D, nj + 1] = np.asarray(bk, np.float32).reshape(D)
    c32[0:D, nj + 2] = np.asarray(bv, np.float32).reshape(D)

    in_maps = []
    for c in range(N_CORES):
        kp = keeps[c]
        nk = len(kp)
        kc = np.zeros((sk2, E), np.float16)
        vc = np.zeros((sk2, E), np.float16)
        kc[0:nk] = key[c][kp].astype(np.float16)
        vc[0:nk] = value[c][kp].astype(np.float16)
        c32c = c32.copy()
        mb = np.full(sk2, np.float32(MASK_NEG))
        mb[0:nk] = 0.0
        c32c[:, 0:nj] = mb.reshape(nj, P).T
        in_maps.append({
            "qst": np.ascontiguousarray(query[c].T).astype(np.float16),
            "kst": np.ascontiguousarray(kc.T),
            "vst": np.ascontiguousarray(vc.T),
            "c16": w16,
            "c32": np.ascontiguousarray(c32c),
        })

    nc = _get_compiled(sk2)
    res = run_bass_kernel_spmd(nc, in_maps, core_ids=list(range(N_CORES)))
    LAST_RESULTS = res
    return np.stack([res.results[c]["out"] for c in range(N_CORES)], axis=0)


# revision 9
# speedup vs baseline: 1.3737x; 1.1585x over previous
"""Self-contained Trainium2 Bass kernel for a single attention head.

Problem: B=8, S=2048, E=1024, D=64 (fp32 in/out).
  q = query @ Wq.T + bq ; k, v likewise
  out = softmax(mask(q @ k.T / sqrt(D))) @ v
  mask = query_mask[:, :, None] * key_mask[:, None, :]; query_mask is all-ones
  per the problem spec (fill="ones").

Sharding: pure data-parallel, one batch element per NeuronCore (8 cores).

Key ideas:
  - fp16 compute with fp32 PSUM accumulation (rel err ~7e-4 vs f32 ref;
    fp16 matmul streams 1 col/cycle vs 4 for fp32).
  - Host compacts away masked key columns (they contribute exactly 0 through
    exp(-inf)); S_k shrinks from 2048 to ~1100, padded to a multiple of 128.
    Pad columns get mask bias -30000 -> exp underflows to exactly 0.
  - Everything transposed so contractions sit on SBUF partitions and softmax's
    key dim sits on partitions: the key mask becomes a per-partition bias on
    the ACT exp (func(scale*x + bias)), and the softmax denominator falls out
    of the AV matmul as a 65th output row (X = [v | ones]).
  - No row-max subtraction: scores/sqrt(D) stay within +-~6, exp <= ~300.
  - Scores matmuls are K=64: row-packed two key-tiles per pass via
    tile_position (0,0)/(64,0) with qT/kT duplicated into partitions 64-127.
  - 4 large staged input DMAs issued on the (otherwise idle) GpSimd SWDGE
    path; tiny const + output DMAs on the Sync HWDGE ring.
"""

from contextlib import ExitStack

import numpy as np

import concourse.bass as bass
import concourse.mybir as mybir
import concourse.tile as tile
from concourse import bacc
from concourse.bass_utils import run_bass_kernel_spmd
from concourse.masks import make_identity

FP16 = mybir.dt.float16
F32 = mybir.dt.float32

N_CORES = 8
B, S, E, D = 8, 2048, 1024, 64
P = 128
NE = E // P            # 8 contraction tiles
NH = 2                 # i halves (PSUM capacity)
HI = S // NH           # 1024 query positions per half
NC = 512               # matmul free-dim chunk (one PSUM bank of f32)
SCALE = 1.0 / np.sqrt(np.float32(D))
MASK_NEG = -30000.0


def _chunks(total, step):
    out = []
    o = 0
    while o < total:
        out.append((o, min(step, total - o)))
        o += step
    return out


def _build(tc: tile.TileContext, ins: dict, out_d: bass.AP, ctx, sk2: int):
    nc = tc.nc
    nj = sk2 // P
    qst_d, kst_d, vst_d = ins["qst"], ins["kst"], ins["vst"]
    c16_d, c32_d = ins["c16"], ins["c32"]

    consts = ctx.enter_context(tc.tile_pool(name="consts", bufs=1))
    stage = ctx.enter_context(tc.tile_pool(name="stage", bufs=2))
    proj = ctx.enter_context(tc.tile_pool(name="proj", bufs=1))
    xpool = ctx.enter_context(tc.tile_pool(name="xpool", bufs=16))
    ppool = ctx.enter_context(tc.tile_pool(name="ppool", bufs=16))
    fin = ctx.enter_context(tc.tile_pool(name="fin", bufs=2))
    ps_mm = ctx.enter_context(tc.tile_pool(name="ps_mm", bufs=2, space="PSUM"))
    ps_sm = ctx.enter_context(tc.tile_pool(name="ps_sm", bufs=2, space="PSUM"))
    ps_acc = ctx.enter_context(tc.tile_pool(name="ps_acc", bufs=1, space="PSUM"))

    # --- staged inputs (4 big SWDGE DMAs; e-blocks side by side) --------
    qst = [stage.tile([P, NE * HI], FP16, tag="qst", bufs=2, name=f"qst{i}")
           for i in range(NH)]
    nc.gpsimd.dma_start(
        out=qst[0][:].rearrange("p (e s) -> p e s", e=NE),
        in_=qst_d.rearrange("(e p) s -> p e s", p=P)[:, :, 0:HI])
    kst = stage.tile([P, NE * sk2], FP16, tag="kst", bufs=1)
    nc.gpsimd.dma_start(
        out=kst[:].rearrange("p (e s) -> p e s", e=NE),
        in_=kst_d.rearrange("(e p) s -> p e s", p=P))
    vst = stage.tile([P, NE * sk2], FP16, tag="vst", bufs=1)
    nc.gpsimd.dma_start(
        out=vst[:].rearrange("p (e s) -> p e s", e=NE),
        in_=vst_d.rearrange("(e p) s -> p e s", p=P))
    nc.gpsimd.dma_start(
        out=qst[1][:].rearrange("p (e s) -> p e s", e=NE),
        in_=qst_d.rearrange("(e p) s -> p e s", p=P)[:, :, HI:S])

    # --- constants -------------------------------------------------------
    c16 = consts.tile([P, 3 * NE * D], FP16, tag="c16")
    c32 = consts.tile([P, nj + 3], F32, tag="c32")
    nc.sync.dma_start(out=c16[:], in_=c16_d[:])
    nc.sync.dma_start(out=c32[:], in_=c32_d[:])
    wq = c16[:, 0:NE * D]
    wk = c16[:, NE * D:2 * NE * D]
    wv = c16[:, 2 * NE * D:3 * NE * D]
    mb = c32[:, 0:nj]
    bq = c32[0:D, nj:nj + 1]
    bk = c32[0:D, nj + 1:nj + 2]
    bv = c32[0:D, nj + 2:nj + 3]

    ident16 = consts.tile([P, P], FP16, tag="ident16")
    ident32 = consts.tile([P, P], F32, tag="ident32")
    warm = consts.tile([P, 16], F32, tag="warm")
    make_identity(nc, ident16[:])
    make_identity(nc, ident32[:])
    nc.vector.memset(warm[:], 0.0)
    nc.scalar.activation(warm[:], warm[:], mybir.ActivationFunctionType.Exp)

    # persistent projected tensors (rows 64:128 duplicate rows 0:64 for the
    # row-packed score matmuls)
    qT_sb = proj.tile([P, S], FP16, tag="qT_sb")
    kT_sb = proj.tile([P, sk2], FP16, tag="kT_sb")
    vT_sb = proj.tile([D, sk2], FP16, tag="vT_sb")

    def project(dst, w, bias_ap, src, ncols, c0):
        """dst[0:64, c0:c0+ncols] = (W @ xT)[:, c0:c0+ncols] + bias."""
        for (po, pn) in _chunks(ncols, HI):
            ps = ps_mm.tile([P, HI], F32, tag="ps_mm")
            for (o, n) in _chunks(pn, NC):
                for e in range(NE):
                    nc.tensor.matmul(
                        ps[0:D, o:o + n],
                        w[:, e * D:(e + 1) * D],
                        src[:, e * ncols + po + o:e * ncols + po + o + n],
                        start=(e == 0), stop=(e == NE - 1),
                    )
            nc.vector.tensor_scalar_add(dst[0:D, c0 + po:c0 + po + pn],
                                        ps[0:D, 0:pn], bias_ap)

    # q half 0, k, v, q half 1 (matches DMA arrival order); duplicates into
    # partitions 64:128 are issued per-piece so the h=0 score loop doesn't
    # wait on later DMAs.
    project(qT_sb, wq, bq, qst[0][:], HI, 0)
    nc.sync.dma_start(out=qT_sb[D:P, 0:HI], in_=qT_sb[0:D, 0:HI])
    project(kT_sb, wk, bk, kst[:], sk2, 0)
    nc.sync.dma_start(out=kT_sb[D:P, :], in_=kT_sb[0:D, :])
    project(vT_sb, wv, bv, vst[:], sk2, 0)

    # X[j] = [v rows | ones column]  [128, 65] fp16
    xt = []
    for j in range(nj):
        ps = ps_sm.tile([P, D], FP16, tag="ps_sm")
        nc.tensor.transpose(ps[:], vT_sb[:, j * P:(j + 1) * P],
                            ident16[0:D, 0:D])
        x = xpool.tile([P, D + 1], FP16, tag="x")
        nc.vector.tensor_copy(x[:, 0:D], ps[:])
        nc.vector.memset(x[:, D:D + 1], 1.0)
        xt.append(x)

    project(qT_sb, wq, bq, qst[1][:], HI, HI)
    nc.sync.dma_start(out=qT_sb[D:P, HI:S], in_=qT_sb[0:D, HI:S])

    # --- attention -------------------------------------------------------
    for h in range(NH):
        num = ps_acc.tile([D + 1, HI], F32, tag="num")
        pm = []
        for j0 in range(0, nj, 2):
            pair = [j0] + ([j0 + 1] if j0 + 1 < nj else [])
            pss = []
            for i, j in enumerate(pair):
                ssT = ps_mm.tile([P, HI], F32, tag="ps_mm")
                lo = i * D
                for c in range(HI // NC):
                    nc.tensor.matmul(
                        ssT[:, c * NC:(c + 1) * NC],
                        kT_sb[lo:lo + D, j * P:(j + 1) * P],
                        qT_sb[lo:lo + D, h * HI + c * NC:h * HI + (c + 1) * NC],
                        start=True, stop=True,
                        tile_position=(lo, 0),
                    )
                pss.append(ssT)
            for i, j in enumerate(pair):
                p = ppool.tile([P, HI], FP16, tag="pm")
                nc.scalar.activation(p[:], pss[i][:],
                                     mybir.ActivationFunctionType.Exp,
                                     bias=mb[:, j:j + 1], scale=float(SCALE))
                pm.append(p)
        for j in range(nj):
            for c in range(HI // NC):
                nc.tensor.matmul(
                    num[:, c * NC:(c + 1) * NC],
                    xt[j][:],
                    pm[j][:, c * NC:(c + 1) * NC],
                    start=(j == 0), stop=(j == nj - 1),
                )

        # finalize: transpose 128-col chunks back, normalize, store
        nsb = fin.tile([D + 1, HI], F32, tag="nsb")
        nc.vector.tensor_copy(nsb[:], num[:])
        osb = fin.tile([P, (HI // P) * D], F32, tag="osb")
        for it in range(HI // P):
            pst = ps_sm.tile([P, D + 1], F32, tag="ps_sm")
            nc.tensor.transpose(pst[:], nsb[:, it * P:(it + 1) * P],
                                ident32[0:D + 1, 0:D + 1])
            rec = fin.tile([P, 1], F32, tag="rec")
            nc.vector.reciprocal(rec[:], pst[:, D:D + 1])
            nc.vector.tensor_scalar_mul(osb[:, it * D:(it + 1) * D],
                                        pst[:, 0:D], rec[:])
        nc.sync.dma_start(
            out=out_d[h * HI:(h + 1) * HI, :].rearrange("(t p) d -> p t d", p=P),
            in_=osb[:].rearrange("p (t d) -> p t d", d=D))


_COMPILED = {}


def _get_compiled(sk2: int):
    if sk2 not in _COMPILED:
        nj = sk2 // P
        nc = bacc.Bacc("TRN2", target_bir_lowering=False, debug=False,
                       num_devices=N_CORES)
        ins = {
            "qst": nc.dram_tensor("qst", [E, S], FP16, kind="ExternalInput").ap(),
            "kst": nc.dram_tensor("kst", [E, sk2], FP16, kind="ExternalInput").ap(),
            "vst": nc.dram_tensor("vst", [E, sk2], FP16, kind="ExternalInput").ap(),
            "c16": nc.dram_tensor("c16", [P, 3 * NE * D], FP16,
                                  kind="ExternalInput").ap(),
            "c32": nc.dram_tensor("c32", [P, nj + 3], F32,
                                  kind="ExternalInput").ap(),
        }
        out_d = nc.dram_tensor("out", [S, D], F32, kind="ExternalOutput").ap()
        with tile.TileContext(nc) as tc:
            with ExitStack() as ctx:
                _build(tc, ins, out_d, ctx, sk2)
        nc.compile()
        _COMPILED[sk2] = nc
    return _COMPILED[sk2]


LAST_RESULTS = None


def kernel(query, key, value, query_mask, key_mask, Wq, bq, Wk, bk, Wv, bv):
    global LAST_RESULTS
    query = np.asarray(query, dtype=np.float32)
    key = np.asarray(key, dtype=np.float32)
    value = np.asarray(value, dtype=np.float32)
    key_mask = np.asarray(key_mask)

    # compact masked keys away (they contribute exactly zero)
    keeps = [np.nonzero(key_mask[c] != 0)[0] for c in range(N_CORES)]
    nk_max = max(len(kp) for kp in keeps)
    sk2 = max(P, int(np.ceil(nk_max / P)) * P)
    sk2 = min(sk2, S)
    nj = sk2 // P

    w16 = np.concatenate(
        [np.asarray(w, np.float32).astype(np.float16)
         .reshape(D, NE, P).transpose(2, 1, 0).reshape(P, NE * D)
         for w in (Wq, Wk, Wv)], axis=1)
    c32 = np.zeros((P, nj + 3), np.float32)
    c32[0:D, nj] = np.asarray(bq, np.float32).reshape(D)
    c32[0:D, nj + 1] = np.asarray(bk, np.float32).reshape(D)
    c32[0:D, nj + 2] = np.asarray(bv, np.float32).reshape(D)

    in_maps = []
    for c in range(N_CORES):
        kp = keeps[c]
        nk = len(kp)
        kc = np.zeros((sk2, E), np.float16)
        vc = np.zeros((sk2, E), np.float16)
        kc[0:nk] = key[c][kp].astype(np.float16)
        vc[0:nk] = value[c][kp].astype(np.float16)
        c32c = c32.copy()
        mb = np.full(sk2, np.float32(MASK_NEG))
        mb[0:nk] = 0.0
        c32c[:, 0:nj] = mb.reshape(nj, P).T
        in_maps.append({
            "qst": np.ascontiguousarray(query[c].T).astype(np.float16),
            "kst": np.ascontiguousarray(kc.T),
            "vst": np.ascontiguousarray(vc.T),
            "c16": w16,
            "c32": np.ascontiguousarray(c32c),
        })

    nc = _get_compiled(sk2)
    res = run_bass_kernel_spmd(nc, in_maps, core_ids=list(range(N_CORES)))
    LAST_RESULTS = res
    return np.stack([res.results[c]["out"] for c in range(N_CORES)], axis=0)
